# revision 22
# baseline (speedup 1.0000x reference)
"""Trainium2 Bass kernel for causal multi-head self-attention.

Problem (hardcoded):
    x:      [2, 2048, 1024] f32
    W_qkv:  [1024, 3072] f32   (cols: [q | k | v], each 1024 = 16 heads x 64)
    b_qkv:  [3072] f32
    W_proj: [1024, 1024] f32
    b_proj: [1024] f32
    out:    [2, 2048, 1024] f32

Sharding over 8 NeuronCores: data parallel on batch (2) x tensor parallel on
heads (4 quads of 4 heads). Core c handles batch c//4, heads [4*(c%4), 4*(c%4)+4).
Each core computes its heads' q/k/v projections, causal-softmax attention, and a
partial output projection (its heads' rows of W_proj). Host gather sums the 4
bf16 partials per batch in f32 and adds b_proj.

On-core dataflow (bf16 operands, f32 PSUM accumulation):
  - qkT [512, S]   = W_qk^T @ x^T   (partitions = qkv-col; 4 M-tiles of 128)
  - v_aug [S,4,65] = x @ W_v + ones column per head (softmax denominator)
  - scoresT[sk,sq] = kT.T @ qT per head, exact 128-granular causal truncation
  - expT = ACT exp(0.125 * scores) -> bf16 (no max-subtraction: |s/8| small)
  - diagonal tiles masked on GpSimd (Pool) with a [128,128] triangular mask
  - AV transposed: ps_attn[sq 128, 4 chunks, 65] += expT_chunk^T @ v_aug
    (N=65 per accumulation step: half the PE cycles of the [65, sq] form)
  - normalize per-partition (sq) via DVE reciprocal + broadcast multiply
  - head pair packed side by side [sq, 2*64] -> XBAR DMA transpose -> [128, sq]
  - y tile [sq 128, 512] = sum over 2 pairs: attn_T^T @ W_proj_pair (K=128)
"""

import os
import sys

for _p in ("/opt/trn_rl_repo", "/root/.axon_site/_ro/trn_rl_repo"):
    if os.path.isdir(_p) and _p not in sys.path:
        sys.path.append(_p)

import numpy as np

import concourse.bass as bass
import concourse.mybir as mybir
import concourse.tile as tile
from concourse import library_config

F32 = mybir.dt.float32
BF16 = mybir.dt.bfloat16
AFT = mybir.ActivationFunctionType

B, S, D, H, HD = 2, 2048, 1024, 16, 64
NCORES = 8
NH = 4  # heads per core
SCALE = 1.0 / 8.0  # 1/sqrt(64)


class SplitWaitTileContext(tile.TileContext):
    """This container's walrus rejects >1 sync wait per instruction
    ("Too many sync wait commands"). Split extra waits onto preceding
    same-engine NoOps before the final block lowering."""

    def _lower_ordered_insts(self, ordered):
        for bb_name, insts in list(ordered.items()):
            new = []
            for inst in insts:
                si = inst.sync_info
                if si is not None and si.on_wait and len(si.on_wait) > 1:
                    waits = list(si.on_wait)
                    for w in waits[:-1]:
                        nop = mybir.InstNoOp(
                            name=f"nopw-{self.nc.get_next_instruction_name()}"
                        )
                        nop.engine = inst.engine
                        nop.sync_info = mybir.SyncInfo(on_wait=[w], on_update=[])
                        new.append(nop)
                    inst.sync_info = mybir.SyncInfo(
                        on_wait=[waits[-1]], on_update=list(si.on_update or [])
                    )
                new.append(inst)
            ordered[bb_name] = new
        return super()._lower_ordered_insts(ordered)

    def _drain_and_barrier(self, tick_clock, wait_clock):
        from concourse.vector_clock import ScopedClock

        drain_inst = self.nc.sync.drain()
        wait_clock.add_sem_waits(
            drain_inst.ins, ScopedClock({None: tick_clock.global_clock})
        )
        si = drain_inst.ins.sync_info
        if si is not None and si.on_wait and len(si.on_wait) > 1:
            waits = list(si.on_wait)
            drain_inst.ins.sync_info = mybir.SyncInfo(
                on_wait=[waits[0]], on_update=list(si.on_update or [])
            )
            for w in waits[1:]:
                nop = self.nc.sync.nop(nofuse=True)
                nop.ins.sync_info = mybir.SyncInfo(on_wait=[w], on_update=[])

        self.nc.all_engine_barrier()
        assert self.sems is not None
        popped = self.nc._tile_sem_poison_stack.pop()
        assert popped is self._sem_poison
        self.nc.clear_and_free_semaphores(list(self.sems.allocated().values()))
        self.nc.all_engine_barrier()


def build_nc(S=S, D=D, NH=NH, dbg=False, reps=1):
    """Build the single-core SPMD program."""
    KD = D // 128        # k-chunks of the D contraction
    NM = NH              # qk M-tiles: 2 q tiles then 2 k tiles
    NMQ = NM // 2
    SQB = S // 512       # sq blocks of 512
    NSK = S // 128       # sk tiles of 128
    NB = min(512, D)     # proj output column block size
    ND = D // NB         # proj output column blocks
    NPAIR = NH // 2

    nc = bass.Bass("TRN2", target_bir_lowering=False, debug=False)

    xT_d = nc.dram_tensor("xT", [D, S], BF16, kind="ExternalInput").ap()
    wqk_d = nc.dram_tensor("wqk", [D, NM * 128], BF16, kind="ExternalInput").ap()
    wv_d = nc.dram_tensor("wv", [D, NH * 64], BF16, kind="ExternalInput").ap()
    bqk_d = nc.dram_tensor("bqk", [NM, 128], F32, kind="ExternalInput").ap()
    bvbc_d = nc.dram_tensor("bvbc", [128, NH * 64], F32, kind="ExternalInput").ap()
    wproj_d = nc.dram_tensor(
        "wproj", [128, NPAIR, D], BF16, kind="ExternalInput"
    ).ap()
    mask_d = nc.dram_tensor("masku", [128, 128], BF16, kind="ExternalInput").ap()
    y_d = nc.dram_tensor("y", [S, D], BF16, kind="ExternalOutput").ap()

    with SplitWaitTileContext(nc) as tc:
        with (
            nc.allow_low_precision(reason="bf16 operands; fp32 accum in PSUM"),
            tc.tile_pool(name="stream", bufs=4) as p_stream,
            tc.tile_pool(name="wpool", bufs=1) as p_w,
            tc.tile_pool(name="qkt", bufs=1) as p_qkt,
            tc.tile_pool(name="vaug", bufs=1) as p_vaug,
            tc.tile_pool(name="expp", bufs=4) as p_exp,
            tc.tile_pool(name="attnn", bufs=2) as p_attn_n,
            tc.tile_pool(name="attnT", bufs=4) as p_attn_T,
            tc.tile_pool(name="rcp", bufs=2) as p_rc,
            tc.tile_pool(name="ypool", bufs=4) as p_y,
            tc.tile_pool(name="pmisc", bufs=2, space="PSUM") as p_misc,
            tc.tile_pool(name="ps", bufs=2, space="PSUM") as p_s,
            tc.tile_pool(name="pav", bufs=2, space="PSUM") as p_av,
        ):
          for _rep in range(reps):
            # PE warmup: junk matmuls keep the systolic array ramped while
            # the input DMAs land
            ones_sb = p_w.tile([128, 64], BF16, tag="ones")
            nc.vector.memset(ones_sb[:, :], 1.0)
            # preload the exp table set in the startup window
            expwarm = p_w.tile([1, 1], F32, tag="expwarm")
            nc.scalar.activation(
                expwarm[:, :], ones_sb[0:1, 0:1], AFT.Exp, scale=SCALE
            )
            warm_ps = p_av.tile([128, NH, 65], F32, tag="av")
            for _w in range(28):
                nc.tensor.matmul(
                    warm_ps[0:64, 0, 0:64],
                    lhsT=ones_sb[:, :],
                    rhs=ones_sb[:, :],
                    start=True,
                    stop=True,
                )

            # startup DMA order: everything the first qk matmuls need first
            # (halves so the first matmuls start while second halves land);
            # batched DMAs keep the serialized HWDGE issue path short
            bqk_sb = p_w.tile([128, NM], F32, tag="bqk")
            nc.sync.dma_start(out=bqk_sb[:, :], in_=bqk_d.rearrange("m p -> p m"))

            xT_src = xT_d.rearrange("(c p) s -> p c s", p=128)
            xs0 = p_stream.tile([128, KD, 512], BF16, tag="xs", name="xs0")
            wqk_sb = p_w.tile([128, KD, NM * 128], BF16, tag="wqk")
            wqk_src = wqk_d.rearrange("(c p) n -> p c n", p=128)
            nc.sync.dma_start(out=xs0[:, 0:KD // 2, :], in_=xT_src[:, 0:KD // 2, 0:512])
            nc.sync.dma_start(out=wqk_sb[:, 0:KD // 2, :], in_=wqk_src[:, 0:KD // 2, :])
            nc.sync.dma_start(out=xs0[:, KD // 2:, :], in_=xT_src[:, KD // 2:, 0:512])
            nc.sync.dma_start(out=wqk_sb[:, KD // 2:, :], in_=wqk_src[:, KD // 2:, :])

            wv_sb = p_w.tile([128, KD, NH * 64], BF16, tag="wv")
            wv_src = wv_d.rearrange("(c p) n -> p c n", p=128)
            nc.sync.dma_start(out=wv_sb[:, :, :], in_=wv_src[:, :, :])

            bvbc_sb = p_w.tile([128, NH * 64], F32, tag="bvbc")
            nc.sync.dma_start(out=bvbc_sb[:, :], in_=bvbc_d[:, :])

            mask_sb = p_w.tile([128, 128], BF16, tag="mask")
            nc.sync.dma_start(out=mask_sb[:, :], in_=mask_d[:, :])

            qkT_sb = p_qkt.tile([128, NM, S], BF16, tag="qkt")
            v_aug = p_vaug.tile([128, NSK, NH, 65], BF16, tag="vaug")
            nc.vector.memset(v_aug[:, :, :, 64:65], 1.0)

            def load_xs(j):
                xs = p_stream.tile([128, KD, 512], BF16, tag="xs", name=f"xs{j}")
                nc.sync.dma_start(
                    out=xs[:, :, :],
                    in_=xT_src[:, :, j * 512:(j + 1) * 512],
                )
                return xs

            # prefetch the remaining x blocks + proj weights up front: SBUF
            # is plentiful and this keeps qkv fillers off the DMA wait path
            xs_all = {0: xs0}
            for j in range(1, SQB):
                xs_all[j] = load_xs(j)

            wproj_sb = p_w.tile([128, NPAIR, D], BF16, tag="wproj")
            nc.sync.dma_start(out=wproj_sb[:, :, :], in_=wproj_d[:, :, :])

            def qk_part(j, xs, mp):
                ps_qk = p_misc.tile([128, NB], F32, tag="m")
                for k in range(KD):
                    nc.tensor.matmul(
                        ps_qk[:, :],
                        lhsT=wqk_sb[:, k, mp * 128:(mp + 1) * 128],
                        rhs=xs[:, k, :],
                        start=(k == 0),
                        stop=(k == KD - 1),
                    )
                nc.vector.tensor_scalar_add(
                    qkT_sb[:, mp, j * 512:(j + 1) * 512],
                    ps_qk[:, :],
                    bqk_sb[:, mp:mp + 1],
                )

            def v_part(j, xs, m):
                ps_v = p_misc.tile([128, NB], F32, tag="m")
                for k in range(KD):
                    nc.tensor.matmul(
                        ps_v[:, 0:NH * 64],
                        lhsT=xs[:, k, (m % 4) * 128:(m % 4) * 128 + 128],
                        rhs=wv_sb[:, k, :],
                        start=(k == 0),
                        stop=(k == KD - 1),
                    )
                nc.vector.tensor_add(
                    v_aug[:, m, :, 0:64],
                    ps_v[:, 0:NH * 64].rearrange("p (h c) -> p h c", c=64),
                    bvbc_sb[:, :].rearrange("p (h c) -> p h c", c=64),
                )

            def qkv_parts(j, xs):
                parts = []
                for mp in range(NM):
                    parts.append(lambda mp=mp: qk_part(j, xs, mp))
                for m in range(4 * j, 4 * j + 4):
                    parts.append(lambda m=m: v_part(j, xs, m))
                return parts

            attn_T = {}

            def attention_block(j, fillers=()):
                fillers = list(fillers)
                n_fill = len(fillers)
                npair = 2 * (j + 1)
                seq = [(h, g) for h in range(NH) for g in range(npair)]
                # finish the fillers a bit before the block ends so none are
                # left stranded behind the final softmax/transpose chain
                pump_span = max(1, len(seq) - 3)
                groups_done = 0
                popped = 0

                def pump():
                    # spread fillers over the block's exp groups; emitted
                    # BEFORE each group's AV so the (in-order) PE works on
                    # them while ACT computes the group's exp
                    nonlocal groups_done, popped
                    groups_done += 1
                    want = min(n_fill, (n_fill * groups_done) // pump_span)
                    while popped < want and fillers:
                        fillers.pop(0)()
                        popped += 1

                attn_T[j] = p_attn_T.tile([128, NPAIR, 512], BF16, tag="attnT", name=f"attnT{j}")
                attn_n = None
                ps_attn = None

                def emit_scores(h, g):
                    # pair of sk tiles i=2g, 2g+1; exact causal column
                    # truncation (bf16 keeps 1 cycle/row at any N)
                    member = h % 2
                    qT = qkT_sb[64 * member:64 * member + 64, h // 2, :]
                    kT = qkT_sb[64 * member:64 * member + 64, NMQ + h // 2, :]
                    ps = p_s.tile([128, 2, 512], F32, tag="s")
                    for b in range(2):
                        i = 2 * g + b
                        no = 128 * max(0, i - 4 * j)
                        nc.tensor.matmul(
                            ps[:, b, no:512],
                            lhsT=kT[:, i * 128:(i + 1) * 128],
                            rhs=qT[:, j * 512 + no:(j + 1) * 512],
                            start=True,
                            stop=True,
                        )
                    return ps

                sc_next = emit_scores(0, 0)
                for idx, (h, g) in enumerate(seq):
                    member, pair = h % 2, h // 2
                    if g == 0:
                        if member == 0:
                            attn_n = p_attn_n.tile([128, 4, 128], BF16, tag="attnn")
                        ps_attn = p_av.tile([128, NH, 65], F32, tag="av")
                    ps_sc = sc_next
                    # 1-deep software pipeline ACROSS heads: the next group's
                    # scores (even of the next head) are emitted before this
                    # group's AV so PE keeps feeding ACT
                    if idx + 1 < len(seq):
                        sc_next = emit_scores(*seq[idx + 1])
                    exp_t = p_exp.tile([128, 2, 512], BF16, tag="exp")
                    if g == 2 * j:
                        # diagonal pair 1: b0 full, b1 valid >= 128 (its
                        # cols 0:128 read stale PSUM; never consumed)
                        nc.scalar.activation(
                            exp_t[:, :, :], ps_sc[:, :, :], AFT.Exp, scale=SCALE
                        )
                    elif g == 2 * j + 1:
                        # diagonal pair 2: b0 valid >= 256, b1 >= 384
                        # (b1 cols 256:384 stale; never consumed)
                        nc.scalar.activation(
                            exp_t[:, :, 256:512],
                            ps_sc[:, :, 256:512],
                            AFT.Exp,
                            scale=SCALE,
                        )
                    else:
                        nc.scalar.activation(
                            exp_t[:, :, :], ps_sc[:, :, :], AFT.Exp, scale=SCALE
                        )
                    if g >= 2 * j:
                        # in-tile causal mask of diagonal tiles, off the
                        # DVE/ACT path (Pool is otherwise idle)
                        for b in range(2):
                            c = 2 * g + b - 4 * j
                            nc.gpsimd.tensor_mul(
                                exp_t[:, b, 128 * c:128 * c + 128],
                                exp_t[:, b, 128 * c:128 * c + 128],
                                mask_sb[:, :],
                            )
                    pump()
                    for b in range(2):
                        i = 2 * g + b
                        clo = max(0, i - 4 * j)
                        # diagonal (masked) chunk last: its AV also waits
                        # on the Pool mask and PE executes in order
                        for c in list(range(clo + 1, 4)) + [clo]:
                            nc.tensor.matmul(
                                ps_attn[:, c, :],
                                lhsT=exp_t[:, b, 128 * c:128 * c + 128],
                                rhs=v_aug[:, i, h, :],
                                start=(i == 0),
                                stop=(i == 4 * j + c),
                            )
                    if g == npair - 1:
                        # normalize: denominators live per-partition (sq), so
                        # a [128,1]-scalar broadcast along free dims works
                        rc = p_rc.tile([128, NH], F32, tag="rc")
                        nc.vector.reciprocal(
                            rc[:, :],
                            ps_attn[:, :, 64:65].rearrange("p a b -> p (a b)"),
                        )
                        rc_ap = rc[:, :]
                        rc_bc = bass.AP(
                            tensor=rc_ap.tensor,
                            offset=rc_ap.offset,
                            ap=list(rc_ap.ap) + [[0, 64]],
                        )
                        nc.vector.tensor_mul(
                            attn_n[:, :, 64 * member:64 * member + 64],
                            ps_attn[:, :, 0:64],
                            rc_bc,
                        )
                        if member == 1:
                            # XBAR transpose [sq 128, 2x64] -> [128, sq]: one
                            # blocked-transpose instruction flips all 4
                            # chunks: out[:, c, :] = in[:, 128c:128c+128].T
                            nc.sync.dma_start_transpose(
                                out=attn_T[j][:, pair, :].rearrange(
                                    "p (c f) -> p c f", f=128
                                ),
                                in_=attn_n[:, :, :],
                            )
                for f in fillers:
                    f()

            def proj_part(j, m, split_dma=False):
                o = (m % 4) * 128
                y_sb = p_y.tile([128, D], BF16, tag="y")
                for n in range(ND):
                    ps_y = p_misc.tile([128, NB], F32, tag="m")
                    for p in range(NPAIR):
                        nc.tensor.matmul(
                            ps_y[:, :],
                            lhsT=attn_T[j][:, p, o:o + 128],
                            rhs=wproj_sb[:, p, n * NB:(n + 1) * NB],
                            start=(p == 0),
                            stop=(p == NPAIR - 1),
                        )
                    nc.vector.tensor_copy(y_sb[:, n * NB:(n + 1) * NB], ps_y[:, :])
                    if split_dma:
                        # tail: ship each half as soon as its copy lands
                        nc.sync.dma_start(
                            out=y_d[m * 128:(m + 1) * 128, n * NB:(n + 1) * NB],
                            in_=y_sb[:, n * NB:(n + 1) * NB],
                        )
                if not split_dma:
                    nc.sync.dma_start(
                        out=y_d[m * 128:(m + 1) * 128, :],
                        in_=y_sb[:, :],
                    )

            def proj_parts(j):
                return [
                    (lambda m=m: proj_part(j, m)) for m in range(j * 4, j * 4 + 4)
                ]

            # j=0 prologue: qkv computed up front. The first two qk parts
            # run k-halves interleaved so their first matmuls start as soon
            # as the first xs0/wqk half-DMAs land.
            ps_h = {}
            for mp in range(2):
                ps_h[mp] = p_misc.tile([128, NB], F32, tag="m", name=f"psh{mp}")
                for k in range(KD // 2):
                    nc.tensor.matmul(
                        ps_h[mp][:, :],
                        lhsT=wqk_sb[:, k, mp * 128:(mp + 1) * 128],
                        rhs=xs0[:, k, :],
                        start=(k == 0),
                        stop=False,
                    )
            for mp in range(2):
                for k in range(KD // 2, KD):
                    nc.tensor.matmul(
                        ps_h[mp][:, :],
                        lhsT=wqk_sb[:, k, mp * 128:(mp + 1) * 128],
                        rhs=xs0[:, k, :],
                        start=False,
                        stop=(k == KD - 1),
                    )
                nc.vector.tensor_scalar_add(
                    qkT_sb[:, mp, 0:512],
                    ps_h[mp][:, :],
                    bqk_sb[:, mp:mp + 1],
                )
            parts0 = qkv_parts(0, xs0)
            for part in parts0[2:]:
                part()
            # proj fillers are pushed toward the last (ACT-heaviest) block:
            # the exp load grows with j while qkv filler supply is constant.
            # Two proj parts of the second-to-last block are held back as a
            # dependency-free bridge over the final softmax/transpose chain.
            bridge = []
            proj_sched = {3: [0, 1]} if SQB == 4 else {
                j: [j - 1] for j in range(1, SQB)
            }
            for j in range(SQB):
                fillers = []
                if j + 1 < SQB:
                    fillers += qkv_parts(j + 1, xs_all[j + 1])
                for jp in proj_sched.get(j, []):
                    fillers += proj_parts(jp)
                if j == SQB - 1 and SQB >= 2:
                    late = proj_parts(SQB - 2)
                    fillers += late[:2]
                    bridge = late[2:]
                attention_block(j, fillers)
            for part in bridge:
                part()
            for m in range((SQB - 1) * 4, (SQB - 1) * 4 + 4):
                proj_part(SQB - 1, m, split_dma=True)

    return nc


def make_mask():
    p = np.arange(128)[:, None]
    f = np.arange(128)[None, :]
    return (f >= p)  # [128, 128] valid region in T layout


def make_in_maps(x, W_qkv, b_qkv, W_proj):
    """Per-core input dicts for the full-size problem."""
    import ml_dtypes

    bf = ml_dtypes.bfloat16
    masku = make_mask().astype(bf)
    in_maps = []
    for c in range(NCORES):
        b, q = c // 4, c % 4
        cq = slice(256 * q, 256 * q + 256)
        wqk = np.concatenate([W_qkv[:, cq], W_qkv[:, 1024:2048][:, cq]], axis=1)
        wv = W_qkv[:, 2048:3072][:, cq]
        bqk = np.concatenate([b_qkv[cq], b_qkv[1024:2048][cq]]).reshape(4, 128)
        bvbc = np.broadcast_to(b_qkv[2048:3072][cq], (128, 256))
        wproj = np.ascontiguousarray(
            W_proj[cq, :].reshape(2, 128, 1024).transpose(1, 0, 2)
        )
        in_maps.append(
            {
                "xT": np.ascontiguousarray(x[b].T).astype(bf),
                "wqk": np.ascontiguousarray(wqk).astype(bf),
                "wv": np.ascontiguousarray(wv).astype(bf),
                "bqk": np.ascontiguousarray(bqk),
                "bvbc": np.ascontiguousarray(bvbc),
                "wproj": wproj.astype(bf),
                "masku": masku,
            }
        )
    return in_maps


_NC_CACHE = {}


def _get_nc():
    if "nc" not in _NC_CACHE:
        _NC_CACHE["nc"] = build_nc()
    return _NC_CACHE["nc"]


def run_on_hw(x, W_qkv, b_qkv, W_proj, b_proj, trace=False, **trace_kw):
    from concourse.bass_utils import run_bass_kernel_spmd

    in_maps = make_in_maps(x, W_qkv, b_qkv, W_proj)
    res = run_bass_kernel_spmd(
        _get_nc(), in_maps, core_ids=list(range(NCORES)), trace=trace, **trace_kw
    )
    out = np.empty((B, S, D), dtype=np.float32)
    for b in range(B):
        acc = res.results[4 * b]["y"].astype(np.float32)
        for q in range(1, 4):
            acc = acc + res.results[4 * b + q]["y"].astype(np.float32)
        out[b] = acc + b_proj[None, :]
    return out, res


def kernel(x, W_qkv, b_qkv, W_proj, b_proj):
    x = np.asarray(x, dtype=np.float32)
    W_qkv = np.asarray(W_qkv, dtype=np.float32)
    b_qkv = np.asarray(b_qkv, dtype=np.float32)
    W_proj = np.asarray(W_proj, dtype=np.float32)
    b_proj = np.asarray(b_proj, dtype=np.float32)
    out, _ = run_on_hw(x, W_qkv, b_qkv, W_proj, b_proj, trace=False)
    return out
